# revision 4
# baseline (speedup 1.0000x reference)
"""Trainium2 Bass kernel for nn_AttractorState — sequence-parallel sharding.

Reference computation (per batch b):
    C[b] = sum_t alpha^(S-1-t) * (W @ h_t + bias) outer e_t        (S = 8192)

Refactored:
    G[b]  = H[b]^T @ (w . PE[b])          # [d_model=512, d_model=512], w_t = alpha^(S-1-t)
    C[b]  = W @ G[b]  (+ bias outer r)    # [d_state=512, d_model=512]

Sharding over 8 NeuronCores, per the sequence-parallel hint:
(batch=4) x (seq-half=2).  Core (b, j) owns tokens [j*4096, (j+1)*4096)
at full width and computes the partial state
    C_j[b] = W @ (H_j^T @ (w_j . PE_j))   # [512, 512]
with the GLOBAL decay weights w_t baked in (each shard uses its own global
w slice, so partials combine by plain addition — the decay-scaled
all-reduce of the hint).  The two partials per batch are summed during
host-side unsharding (the pair-sum IS the unshard for sum-sharded
partials; on-device NRT collectives measure 40-70us of entry/serialize
overhead on this 8-core setup — 10x the data cost — so the 1MB/pair
reduction rides the output gather instead).

Per-core HBM traffic: 8.5 MiB (hp interleaved h+decayed-pe rows in bf16,
W^T, bf16 partial out).  G[e,d] accumulates with e on partitions so the
tail needs no transposes.

Schedule notes (v2, from NTFF profile analysis):
- The PE HAM clock gate starts at K=4/8 (1.2 GHz) and only reaches 2.4 GHz
  after ~3.4us of sustained matmul activity.  A run of warmup matmuls on a
  memset scratch tile (into a scratch PSUM bank that is never read) runs
  during the DMA ramp, so every data matmul executes warm at the
  ~216 ns/MM N=512 streaming rate.
- DMA_DIRECT2D triggers cost ~650 ns of issuing-engine time each, so hp
  rides in 12 triggers (front-loaded small chunks, then 4-tile chunks)
  instead of 22.
- PSUM->SBUF copies alternate vector/gpsimd; no scalar activations, which
  drops the 1.3us ACT_TABLE_LOAD from the scalar queue.
- wt is a single DMA; out slices stream per-cs on alternating queues so
  only the last 128 KiB transfer is exposed at the tail.

The bias path (b != 0) needs an extra rank-1 accumulation r = w^T @ PE and
an outer-product matmul; setup_inputs() always produces b == 0, so the
default graph skips it and a bias-capable graph is built only if a nonzero
b ever shows up.
"""

import math
import sys

import numpy as np

for _p in ("/opt/trn_rl_repo", "/opt/trn_rl_repo/concourse"):
    if _p not in sys.path:
        sys.path.append(_p)

# Problem constants (hardcoded per harness contract).
B = 4
S = 8192
D = 512          # d_model
E = 512          # d_state
P = 128          # SBUF partitions
NCORES = 8
SH = S // 2      # 4096 tokens per core
NT = SH // P     # 32 t-tiles per core
HP = 2 * D       # 1024, interleaved h+pew row

# hp DMA chunking: tiles 0-1 split h/pe across both queues (lowest
# latency while the rings ramp), tiles 2-3 whole, then 4-tile chunks.
SPLIT = 2        # tiles DMA'd as separate h/pe halves
SINGLE = 2       # whole-tile DMAs after the split tiles
SIZES = [3, 4, 4, 4, 4, 4, 4, 1]
assert SPLIT + SINGLE + sum(SIZES) == NT
CH = len(SIZES)

N_WARMUP = 9     # ~3.9us of cold-rate warmup matmuls to flip the HAM gate

_GRAPH_CACHE = {}


def _decay_weights():
    # Match reference: alpha = f32(exp(-pi/S)); w = exp((S-1-t) * log(alpha)) in f32.
    alpha = np.float32(math.exp(-math.pi / S))
    t = np.arange(S, dtype=np.float32)
    w = np.exp((np.float32(S - 1.0) - t) * np.log(alpha)).astype(np.float32)
    return w


def _build(bias: bool):
    key = "bias" if bias else "nobias"
    if key in _GRAPH_CACHE:
        return _GRAPH_CACHE[key]

    import concourse.bass as bass  # noqa: F401
    import concourse.mybir as mybir
    import concourse.tile as tile
    from concourse import bacc

    f32 = mybir.dt.float32
    bf16 = mybir.dt.bfloat16

    nc = bacc.Bacc("TRN2", target_bir_lowering=False)

    hp_ext = nc.declare_dram_parameter("hp", [SH, HP], bf16, isOutput=False)
    wt_ext = nc.declare_dram_parameter("wt", [D, E], bf16, isOutput=False)  # W^T
    b_ext = nc.declare_dram_parameter("b", [E], f32, isOutput=False)
    out_ext = nc.declare_dram_parameter("out", [E, D], bf16, isOutput=True)

    hp_re = hp_ext.ap().rearrange("(n p) d -> p n d", p=P)
    wt_re = wt_ext.ap().rearrange("(c p) s -> p c s", p=P)

    starts = [sum(SIZES[:j]) for j in range(CH)]

    with tile.TileContext(nc) as tc:
        with (
            tc.tile_pool(name="sb", bufs=1) as io,
            tc.tile_pool(name="acc", bufs=1, space="PSUM") as acc_pool,
        ):
            consts = io

            # ---- PE warmup: matmuls on a memset scratch tile into a
            # scratch PSUM bank (never read).  Keeps the PE busy through
            # the HAM activity window while the hp DMAs ramp, so the data
            # matmuls all run at the warm 2.4 GHz streaming rate. ----
            warm_sb = io.tile([P, E], bf16, tag="warm_sb")
            nc.gpsimd.memset(warm_sb[:], 0.0)
            warm_ps = acc_pool.tile([P, E], f32, tag="warm")
            for i in range(N_WARMUP):
                nc.tensor.matmul(
                    warm_ps[:],
                    warm_sb[:, 0:P],
                    warm_sb[:],
                    start=(i == 0),
                    stop=(i == N_WARMUP - 1),
                )

            # ---- input stream: one resident hp slab, chunked DMA ----
            hp_t = io.tile([P, NT, HP], bf16)
            for n in range(SPLIT):
                nc.sync.dma_start(hp_t[:, n, 0:D], hp_re[:, n, 0:D])
                nc.scalar.dma_start(hp_t[:, n, D:HP], hp_re[:, n, D:HP])
            for n in range(SPLIT, SPLIT + SINGLE):
                eng = nc.sync if (n % 2 == 0) else nc.scalar
                eng.dma_start(hp_t[:, n, :], hp_re[:, n, :])
            for j in range(CH):
                n0, w = SPLIT + SINGLE + starts[j], SIZES[j]
                eng = nc.sync if (j % 2 == 0) else nc.scalar
                eng.dma_start(hp_t[:, n0:n0 + w, :], hp_re[:, n0:n0 + w, :])

            # ---- constants (sync queue tail; needed only at the end) ----
            wt_sb = consts.tile([P, 4, E], bf16)   # wt_sb[p,c,s] = W[s, c*128+p]
            nc.sync.dma_start(wt_sb[:], wt_re[:])
            if bias:
                b_sb = consts.tile([1, E], f32)
                nc.sync.dma_start(b_sb[:], b_ext.ap().unsqueeze(0))
                b_bf = consts.tile([1, E], bf16)
                nc.vector.tensor_copy(b_bf[:], b_sb[:])
                ones_sb = consts.tile([P, 1], bf16)
                nc.vector.memset(ones_sb[:], 1.0)

            # ---- G (/ r) accumulation over this core's 4096 tokens ----
            # G[e, d] += h[t, e-slice]^T @ pew[t, :], e on partitions
            g_ps = [
                acc_pool.tile([P, E], f32, tag=f"g{k}", name=f"g{k}")
                for k in range(4)
            ]
            if bias:
                r_ps = acc_pool.tile([1, E], f32, tag="r")

            def mm_tile(n, first, last):
                for k in range(4):
                    nc.tensor.matmul(
                        g_ps[k][:],
                        hp_t[:, n, k * P:(k + 1) * P],
                        hp_t[:, n, D:HP],
                        start=first,
                        stop=last,
                    )
                if bias:
                    nc.tensor.matmul(
                        r_ps[:],
                        ones_sb[:],
                        hp_t[:, n, D:HP],
                        start=first,
                        stop=last,
                    )

            for n in range(NT - SIZES[-1]):
                mm_tile(n, first=(n == 0), last=False)
            # final chunk k-grouped: g_ps[k] finalize in order so their
            # PSUM->SBUF copies overlap the remaining matmuls
            nL, wL = NT - SIZES[-1], SIZES[-1]
            for k in range(4):
                for i in range(wL):
                    nc.tensor.matmul(
                        g_ps[k][:],
                        hp_t[:, nL + i, k * P:(k + 1) * P],
                        hp_t[:, nL + i, D:HP],
                        start=False,
                        stop=(i == wL - 1),
                    )
            if bias:
                for i in range(wL):
                    nc.tensor.matmul(
                        r_ps[:],
                        ones_sb[:],
                        hp_t[:, nL + i, D:HP],
                        start=False,
                        stop=(i == wL - 1),
                    )

            # ---- G -> SBUF (bf16); no transposes needed ----
            g_sb = consts.tile([P, 4, E], bf16)   # g_sb[p,c,d] = G[c*128+p, d]
            for k in range(4):
                nc.vector.tensor_copy(g_sb[:, k, :], g_ps[k][:])
            if bias:
                rred_bf = consts.tile([1, E], bf16)
                nc.vector.tensor_copy(rred_bf[:], r_ps[:])

            # ---- partial C = W @ G (+ b outer r), bf16 out ----
            for cs in range(4):
                c_ps = acc_pool.tile(
                    [P, E], f32, tag=f"cps{cs % 2}", name=f"cps{cs}"
                )
                for ce in range(4):
                    nc.tensor.matmul(
                        c_ps[:],
                        wt_sb[:, ce, cs * P:(cs + 1) * P],
                        g_sb[:, ce, :],
                        start=(ce == 0),
                        stop=(not bias and ce == 3),
                    )
                if bias:
                    nc.tensor.matmul(
                        c_ps[:],
                        b_bf[0:1, cs * P:(cs + 1) * P],
                        rred_bf[:],
                        start=False,
                        stop=True,
                    )
                c_sb = io.tile([P, E], bf16, tag="csb", bufs=4)
                nc.vector.tensor_copy(c_sb[:], c_ps[:])
                deng = nc.sync if (cs % 2 == 0) else nc.scalar
                deng.dma_start(out_ext[cs * P:(cs + 1) * P, :], c_sb[:])

    nc.compile()
    _GRAPH_CACHE[key] = nc
    return nc


def _in_maps(hidden_states, positional_encodings, W, b):
    import ml_dtypes

    bf16 = ml_dtypes.bfloat16
    w_full = _decay_weights()[:, None]  # constant decay, folded into pe staging
    wt = np.ascontiguousarray(np.asarray(W, dtype=np.float32).T.astype(bf16))
    b_c = np.ascontiguousarray(b, dtype=np.float32)
    maps = []
    for c in range(NCORES):
        bi, sj = c // 2, c % 2
        lo, hi = sj * SH, (sj + 1) * SH
        hp = np.empty((SH, HP), dtype=bf16)
        hp[:, 0:D] = np.asarray(
            hidden_states[bi, lo:hi], dtype=np.float32
        ).astype(bf16)
        hp[:, D:HP] = (
            np.asarray(positional_encodings[bi, lo:hi], dtype=np.float32)
            * w_full[lo:hi]
        ).astype(bf16)
        maps.append({"hp": hp, "wt": wt, "b": b_c})
    return maps


def _assemble(results):
    # pair-sum is the unshard for sum-sharded partial states
    out = np.empty((B, E, D), dtype=np.float32)
    for bi in range(B):
        out[bi] = results[2 * bi]["out"].astype(np.float32) + results[
            2 * bi + 1
        ]["out"].astype(np.float32)
    return out


def run(hidden_states, positional_encodings, W, b, trace=False, **trace_kwargs):
    from concourse.bass_utils import run_bass_kernel_spmd

    nc = _build(bias=bool(np.any(np.asarray(b) != 0)))
    maps = _in_maps(hidden_states, positional_encodings, W, b)
    res = run_bass_kernel_spmd(
        nc, maps, core_ids=list(range(NCORES)), trace=trace, **trace_kwargs
    )
    return _assemble(res.results), res


def kernel(hidden_states, positional_encodings, W, b):
    out, _ = run(hidden_states, positional_encodings, W, b, trace=False)
    return out


# revision 5
# speedup vs baseline: 1.0768x; 1.0768x over previous
"""Trainium2 Bass kernel for nn_AttractorState — sequence-parallel sharding.

Reference computation (per batch b):
    C[b] = sum_t alpha^(S-1-t) * (W @ h_t + bias) outer e_t        (S = 8192)

Refactored:
    G[b]  = H[b]^T @ (w . PE[b])          # [d_model=512, d_model=512], w_t = alpha^(S-1-t)
    C[b]  = W @ G[b]  (+ bias outer r)    # [d_state=512, d_model=512]

Sharding over 8 NeuronCores, per the sequence-parallel hint:
(batch=4) x (seq-half=2).  Core (b, j) owns tokens [j*4096, (j+1)*4096)
at full width and computes the partial state
    C_j[b] = W @ (H_j^T @ (w_j . PE_j))   # [512, 512]
with the GLOBAL decay weights w_t baked in (each shard uses its own global
w slice, so partials combine by plain addition — the decay-scaled
all-reduce of the hint).  The two partials per batch are summed during
host-side unsharding (the pair-sum IS the unshard for sum-sharded
partials; on-device NRT collectives measure 40-70us of entry/serialize
overhead on this 8-core setup — 10x the data cost — so the 1MB/pair
reduction rides the output gather instead).

Per-core HBM traffic: 8.5 MiB (hp interleaved h+decayed-pe rows in bf16,
W^T, bf16 partial out).  G[e,d] accumulates with e on partitions so the
tail needs no transposes.

Schedule notes (v3, from NTFF profile analysis):
- All DRAM tensors are host-staged PARTITION-MAJOR ([128, ...] with each
  partition's bytes contiguous), so every DMA is one large contiguous
  descriptor per partition.  The HWDGE generates descriptors at ~10ns
  each; with row-wise (2 KiB) descriptors that caps a ring at ~200 GB/s
  and dominates the ramp.  One descriptor per partition per chunk makes
  descriptor generation negligible.
- The PE HAM clock gate starts at K=4/8 (1.2 GHz) and reaches 2.4 GHz
  only after ~3.4us of sustained matmul activity.  Six warmup matmuls on
  a vector-memset scratch tile (into a PSUM bank later reused for C)
  run during the DMA ramp, so data matmuls execute at the warm
  ~216 ns/MM N=512 streaming rate from the first tile.
- hp rides in 13 triggers: singles/pairs up front (low latency while the
  rings ramp), then 4-tile chunks.
- W@G runs ce-major so it only needs g_sb[ce] as the PE reaches round ce
  — no PE stall on the g3 PSUM->SBUF copy.
- PSUM->SBUF copies alternate vector/scalar; output slices pair up into
  two DMAs on alternating queues so only the last 256 KiB is exposed.

The bias path (b != 0) needs an extra rank-1 accumulation r = w^T @ PE and
an outer-product matmul; setup_inputs() always produces b == 0, so the
default graph skips it and a bias-capable graph is built only if a nonzero
b ever shows up.
"""

import math
import sys

import numpy as np

for _p in ("/opt/trn_rl_repo", "/opt/trn_rl_repo/concourse"):
    if _p not in sys.path:
        sys.path.append(_p)

# Problem constants (hardcoded per harness contract).
B = 4
S = 8192
D = 512          # d_model
E = 512          # d_state
P = 128          # SBUF partitions
NCORES = 8
SH = S // 2      # 4096 tokens per core
NT = SH // P     # 32 t-tiles per core
HP = 2 * D       # 1024, interleaved h+pew row

# hp chunk schedule: (start, width) pairs, alternating sync/scalar queues.
CHUNKS = [
    (0, 1), (1, 1), (2, 2), (4, 2), (6, 2), (8, 2), (10, 2), (12, 2),
    (14, 4), (18, 4), (22, 4), (26, 4), (30, 2),
]
assert sum(w for _, w in CHUNKS) == NT
assert [s for s, _ in CHUNKS] == [sum(w for _, w in CHUNKS[:i]) for i in range(len(CHUNKS))]
WL = CHUNKS[-1][1]   # last chunk is k-grouped for staggered g finalization

N_WARMUP = 6     # ~3us of cold-rate warmup matmuls to flip the HAM gate

_GRAPH_CACHE = {}


def _decay_weights():
    # Match reference: alpha = f32(exp(-pi/S)); w = exp((S-1-t) * log(alpha)) in f32.
    alpha = np.float32(math.exp(-math.pi / S))
    t = np.arange(S, dtype=np.float32)
    w = np.exp((np.float32(S - 1.0) - t) * np.log(alpha)).astype(np.float32)
    return w


def _build(bias: bool):
    key = "bias" if bias else "nobias"
    if key in _GRAPH_CACHE:
        return _GRAPH_CACHE[key]

    import concourse.bass as bass  # noqa: F401
    import concourse.mybir as mybir
    import concourse.tile as tile
    from concourse import bacc

    f32 = mybir.dt.float32
    bf16 = mybir.dt.bfloat16

    nc = bacc.Bacc("TRN2", target_bir_lowering=False)

    # All DRAM tensors partition-major: per-partition bytes contiguous.
    hp_ext = nc.declare_dram_parameter("hp", [P, NT, HP], bf16, isOutput=False)
    wt_ext = nc.declare_dram_parameter("wt", [P, 4, E], bf16, isOutput=False)
    b_ext = nc.declare_dram_parameter("b", [E], f32, isOutput=False)
    out_ext = nc.declare_dram_parameter("out", [P, 4, D], bf16, isOutput=True)

    with tile.TileContext(nc) as tc:
        with (
            tc.tile_pool(name="sb", bufs=1) as io,
            tc.tile_pool(name="acc", bufs=1, space="PSUM") as acc_pool,
        ):
            consts = io

            # ---- PE warmup: matmuls on a memset scratch tile into a PSUM
            # bank later reused for C (never read from the warmup).  Keeps
            # the PE busy through the HAM activity window while the hp
            # DMAs ramp, so the data matmuls all run at 2.4 GHz. ----
            warm_sb = io.tile([P, E], bf16, tag="warm_sb")
            nc.vector.memset(warm_sb[:], 0.0)
            warm_ps = acc_pool.tile([P, E], f32, tag="cps0", name="warm")
            for i in range(N_WARMUP):
                nc.tensor.matmul(
                    warm_ps[:],
                    warm_sb[:, 0:P],
                    warm_sb[:],
                    start=(i == 0),
                    stop=(i == N_WARMUP - 1),
                )

            # ---- input stream: one resident hp slab, chunked DMA ----
            hp_t = io.tile([P, NT, HP], bf16)
            for j, (n0, w) in enumerate(CHUNKS):
                eng = nc.sync if (j % 2 == 0) else nc.scalar
                eng.dma_start(hp_t[:, n0:n0 + w, :], hp_ext.ap()[:, n0:n0 + w, :])

            # ---- constants (sync queue tail; needed only at the end) ----
            wt_sb = consts.tile([P, 4, E], bf16)   # wt_sb[p,c,s] = W[s, c*128+p]
            nc.sync.dma_start(wt_sb[:], wt_ext.ap())
            if bias:
                b_sb = consts.tile([1, E], f32)
                nc.sync.dma_start(b_sb[:], b_ext.ap().unsqueeze(0))
                b_bf = consts.tile([1, E], bf16)
                nc.vector.tensor_copy(b_bf[:], b_sb[:])
                ones_sb = consts.tile([P, 1], bf16)
                nc.vector.memset(ones_sb[:], 1.0)

            # ---- G (/ r) accumulation over this core's 4096 tokens ----
            # G[e, d] += h[t, e-slice]^T @ pew[t, :], e on partitions
            g_ps = [
                acc_pool.tile([P, E], f32, tag=f"g{k}", name=f"g{k}")
                for k in range(4)
            ]
            if bias:
                r_ps = acc_pool.tile([1, E], f32, tag="r")

            for n in range(NT - WL):
                for k in range(4):
                    nc.tensor.matmul(
                        g_ps[k][:],
                        hp_t[:, n, k * P:(k + 1) * P],
                        hp_t[:, n, D:HP],
                        start=(n == 0),
                        stop=False,
                    )
                if bias:
                    nc.tensor.matmul(
                        r_ps[:],
                        ones_sb[:],
                        hp_t[:, n, D:HP],
                        start=(n == 0),
                        stop=False,
                    )
            # final chunk k-grouped: g_ps[k] finalize in order so their
            # PSUM->SBUF copies overlap the remaining matmuls
            nL = NT - WL
            for k in range(4):
                for i in range(WL):
                    nc.tensor.matmul(
                        g_ps[k][:],
                        hp_t[:, nL + i, k * P:(k + 1) * P],
                        hp_t[:, nL + i, D:HP],
                        start=False,
                        stop=(i == WL - 1),
                    )
            if bias:
                for i in range(WL):
                    nc.tensor.matmul(
                        r_ps[:],
                        ones_sb[:],
                        hp_t[:, nL + i, D:HP],
                        start=False,
                        stop=(i == WL - 1),
                    )

            # ---- G -> SBUF (bf16); no transposes needed ----
            g_sb = consts.tile([P, 4, E], bf16)   # g_sb[p,c,d] = G[c*128+p, d]
            for k in range(4):
                nc.vector.tensor_copy(g_sb[:, k, :], g_ps[k][:])
            if bias:
                rred_bf = consts.tile([1, E], bf16)
                nc.vector.tensor_copy(rred_bf[:], r_ps[:])

            # ---- partial C = W @ G (+ b outer r), bf16 out ----
            # ce-major: round ce only needs g_sb[ce], so the PE never
            # waits on the tail PSUM->SBUF copies.
            c_ps = [
                acc_pool.tile([P, E], f32, tag=f"cps{cs}", name=f"cps{cs}")
                for cs in range(4)
            ]
            for ce in range(4):
                for cs in range(4):
                    nc.tensor.matmul(
                        c_ps[cs][:],
                        wt_sb[:, ce, cs * P:(cs + 1) * P],
                        g_sb[:, ce, :],
                        start=(ce == 0),
                        stop=(not bias and ce == 3),
                    )
            if bias:
                for cs in range(4):
                    nc.tensor.matmul(
                        c_ps[cs][:],
                        b_bf[0:1, cs * P:(cs + 1) * P],
                        rred_bf[:],
                        start=False,
                        stop=True,
                    )
            c_lo = io.tile([P, 2, E], bf16, tag="clo")
            c_hi = io.tile([P, 2, E], bf16, tag="chi")
            nc.vector.tensor_copy(c_lo[:, 0, :], c_ps[0][:])
            nc.scalar.activation(
                c_lo[:, 1, :], c_ps[1][:], mybir.ActivationFunctionType.Copy
            )
            nc.sync.dma_start(out_ext.ap()[:, 0:2, :], c_lo[:])
            nc.vector.tensor_copy(c_hi[:, 0, :], c_ps[2][:])
            nc.scalar.activation(
                c_hi[:, 1, :], c_ps[3][:], mybir.ActivationFunctionType.Copy
            )
            nc.scalar.dma_start(out_ext.ap()[:, 2:4, :], c_hi[:])

    nc.compile()
    _GRAPH_CACHE[key] = nc
    return nc


def _in_maps(hidden_states, positional_encodings, W, b):
    import ml_dtypes

    bf16 = ml_dtypes.bfloat16
    w_full = _decay_weights()[:, None]  # constant decay, folded into pe staging
    # wt partition-major: wt[p, c, s] = W[s, c*128+p]
    wt = np.ascontiguousarray(
        np.asarray(W, dtype=np.float32).T.astype(bf16)
        .reshape(4, P, E).transpose(1, 0, 2)
    )
    b_c = np.ascontiguousarray(b, dtype=np.float32)
    maps = []
    for c in range(NCORES):
        bi, sj = c // 2, c % 2
        lo, hi = sj * SH, (sj + 1) * SH
        # hp partition-major: hp[p, n, 0:D] = h[n*128+p], hp[p, n, D:] = pew[n*128+p]
        hp = np.empty((P, NT, HP), dtype=bf16)
        hp[:, :, 0:D] = (
            np.asarray(hidden_states[bi, lo:hi], dtype=np.float32)
            .astype(bf16).reshape(NT, P, D).transpose(1, 0, 2)
        )
        hp[:, :, D:HP] = (
            (np.asarray(positional_encodings[bi, lo:hi], dtype=np.float32)
             * w_full[lo:hi])
            .astype(bf16).reshape(NT, P, D).transpose(1, 0, 2)
        )
        maps.append({"hp": hp, "wt": wt, "b": b_c})
    return maps


def _assemble(results):
    # pair-sum is the unshard for sum-sharded partial states;
    # out is partition-major: out[p, cs, d] = C[cs*128+p, d]
    out = np.empty((B, E, D), dtype=np.float32)
    for bi in range(B):
        c = results[2 * bi]["out"].astype(np.float32) + results[
            2 * bi + 1
        ]["out"].astype(np.float32)
        out[bi] = c.transpose(1, 0, 2).reshape(E, D)
    return out


def run(hidden_states, positional_encodings, W, b, trace=False, **trace_kwargs):
    from concourse.bass_utils import run_bass_kernel_spmd

    nc = _build(bias=bool(np.any(np.asarray(b) != 0)))
    maps = _in_maps(hidden_states, positional_encodings, W, b)
    res = run_bass_kernel_spmd(
        nc, maps, core_ids=list(range(NCORES)), trace=trace, **trace_kwargs
    )
    return _assemble(res.results), res


def kernel(hidden_states, positional_encodings, W, b):
    out, _ = run(hidden_states, positional_encodings, W, b, trace=False)
    return out
